# revision 43
# baseline (speedup 1.0000x reference)
"""Trainium2 Bass kernel for a 2-head MultiHeadAttn + residual + LayerNorm block.

Problem shapes (hardcoded):
  x:      [8, 2048, 384] f32      attn_mask: [8, 2048] bool (True = attend)
  qkv_w:  [384, 384] f32          qkv_b: [384] f32
  o_w:    [128, 384] f32          ln_g, ln_b: [384] f32
  out:    [8, 2048, 384] f32

Sharding: data-parallel over batch — 8 batch elements, one per NeuronCore.
Each core runs the identical program (SPMD) on its own batch slice.

Per-core dataflow (S=2048, D_model=384, H=2, Dh=64):
  1. Weights DMA first (qkv needs them immediately), then xT [128, 3, 2048]
     bf16 via DMA-transpose split across the two HWDGE queues (sync +
     scalar), then the residual copy xr [128, 16, 384] bf16.
  2. qkvT = (x @ qkv_w).T as [128 j, 2048 s] in fp8e4; j-tile order V,K,Q
     so V-prep (per-head fp8 Vtil with a ones/mask column appended for the
     softmax denominator) overlaps the K/Q projections.
  3. Attention in 4 units (head h x q-half): per k-chunk c, scores^T
     [128 k, 1024 q] on PE (fp8 Q/K); exp alternates per chunk between ACT
     (exact Exp, fp8e4 out) and DVE (Schraudolph bit-trick:
     round(s*SCALE*8/ln2 + 56) as int8 = e4m3 bits, ~5% sawtooth error that
     cancels in softmax normalization); pv uses fp8 DoubleRow matmuls
     (contraction 256 = two k-chunks per pass) accumulating [65, 1024]
     (row 64 = denominator).
  4. Each unit's tail (denominator transpose, reciprocal, o-projection,
     residual combine, LayerNorm) is interleaved piecewise into the NEXT
     unit's attention loop so PE/ACT/DVE all stay dense.  rstd comes from a
     batched Quake rsqrt + 2 Newton steps on DVE (keeps Sqrt off ACT,
     avoiding Exp<->Sqrt activation-table reload thrash).
"""

import os
import sys

import ml_dtypes
import numpy as np

for _p in ("/opt/trn_rl_repo", "/root/.axon_site/_ro/trn_rl_repo"):
    if os.path.isdir(_p) and _p not in sys.path:
        sys.path.insert(0, _p)

import concourse.bass as bass  # noqa: E402
import concourse.tile as tile  # noqa: E402
from concourse import bacc  # noqa: E402
from concourse import mybir  # noqa: E402
from concourse.bass_utils import run_bass_kernel_spmd  # noqa: E402

FP = mybir.dt.float32
BF = mybir.dt.bfloat16
I16 = mybir.dt.int16
I8 = mybir.dt.int8
I32 = mybir.dt.int32
F8 = mybir.dt.float8e4
AF = mybir.ActivationFunctionType
OP = mybir.AluOpType
DROW = mybir.MatmulPerfMode.DoubleRow

B, S, DM = 8, 2048, 384
H, DH = 2, 64
INNER = H * DH  # 128
P = 128
SC = S // P  # 16 s-chunks of 128
DC = DM // P  # 3 model-dim chunks of 128
NQ = S // 512  # 4 q-tiles of 512
LN_EPS = 1e-3
N_CORES = 8
SCALE = 1.0 / (DH**0.5)
# Schraudolph exp in fp8e4-space: exp(SCALE*s) ~ bitcast_e4m3(int8(
#   round(SCALE*s * 8/ln2 + 7*8)))
A_SCHR8 = SCALE * 8.0 / float(np.log(2.0))
B_SCHR8 = 7.0 * 8.0
QUAKE_C = 0x5F3759DF  # rsqrt seed constant


def _build(has_mask: bool, has_bias: bool, has_affine: bool) -> bass.Bass:
    nc = bacc.Bacc(
        "TRN2", target_bir_lowering=False, debug=False, num_devices=N_CORES
    )

    xb_d = nc.dram_tensor("x_bf", [S, DM], BF, kind="ExternalInput")
    w_d = nc.dram_tensor("qkv_w_bf", [DM, 3 * INNER], BF, kind="ExternalInput")
    ow_d = nc.dram_tensor("o_w_bf", [INNER, DM], BF, kind="ExternalInput")
    mask_d = bias_d = g_d = b_d = None
    if has_mask:
        mask_d = nc.dram_tensor("mask_f", [S], FP, kind="ExternalInput")
    if has_bias:
        bias_d = nc.dram_tensor("qkv_b", [3 * INNER], FP, kind="ExternalInput")
    if has_affine:
        g_d = nc.dram_tensor("ln_g", [DM], FP, kind="ExternalInput")
        b_d = nc.dram_tensor("ln_b", [DM], FP, kind="ExternalInput")
    y_d = nc.dram_tensor("y", [S, DM], FP, kind="ExternalOutput")

    with tile.TileContext(nc) as tc:
        with tc.tile_pool(name="singles", bufs=1) as sg:
            # ---- input DMAs; weights first (qkv needs them immediately),
            # then xT transposes split across both HWDGE queues ----
            tq = [nc.sync, nc.scalar]
            w_sb = sg.tile([P, DC, 3 * INNER], BF, tag="w_sb")
            nc.sync.dma_start(w_sb, w_d.rearrange("(dc dp) j -> dp dc j", dp=P))
            ow_sb = sg.tile([DH, H, DM], BF, tag="ow_sb")
            nc.scalar.dma_start(ow_sb, ow_d.rearrange("(h d) m -> d h m", d=DH))
            xT = sg.tile([P, DC, S], BF, tag="xT")
            for st in range(NQ):
                for dc in range(DC):
                    tq[(st * DC + dc) % 2].dma_start_transpose(
                        xT[:, dc, st * 512 : (st + 1) * 512],
                        xb_d[st * 512 : (st + 1) * 512, dc * P : (dc + 1) * P],
                    )
            xr = sg.tile([P, SC, DM], BF, tag="xr")
            xr_src = xb_d.rearrange("(c p) d -> p c d", p=P)
            for c in range(SC):
                tq[c % 2].dma_start(xr[:, c, :], xr_src[:, c, :])

            from concourse.masks import make_identity
            ident = sg.tile([P, P], BF, tag="ident")
            make_identity(nc, ident)
            one_f = sg.tile([P, 1], FP, tag="one_f")
            nc.vector.memset(one_f, 1.0)
            ctile = sg.tile([P, 8], I32, tag="ctile")
            nc.vector.memset(ctile, QUAKE_C)

            mask_sb = bias_sb = g_sb = b_sb = None
            if mask_d is not None:
                mask_sb = sg.tile([P, SC], FP, tag="mask_sb")
                nc.sync.dma_start(mask_sb, mask_d.rearrange("(c p) -> p c", p=P))
            if bias_d is not None:
                bias_sb = sg.tile([P, 3], FP, tag="bias_sb")
                nc.sync.dma_start(bias_sb, bias_d.rearrange("(jt p) -> p jt", p=P))
            if g_d is not None and b_d is not None:
                g_sb = sg.tile([P, DM], FP, tag="g_sb")
                b_sb = sg.tile([P, DM], FP, tag="b_sb")
                nc.gpsimd.dma_start(g_sb, g_d[None, :].to_broadcast((P, DM)))
                nc.gpsimd.dma_start(b_sb, b_d[None, :].to_broadcast((P, DM)))

            qkvT = sg.tile([P, 2, S], F8, tag="qkvT")  # j-tile: 0=Q^T 1=K^T
            vT = sg.tile([P, S], BF, tag="vT")
            vt = [
                sg.tile([P, SC, 80], F8, tag=f"vt{h}", name=f"vt{h}")
                for h in range(H)
            ]
            attnT = [
                sg.tile([DH, S], BF, tag=f"attnT{h}", name=f"attnT{h}")
                for h in range(H)
            ]
            stage = sg.tile([P, H * S], FP, tag="stage")
            r_sb = sg.tile([P, H * SC], FP, tag="r_sb")
            t0 = sg.tile([P, SC, DM], FP, tag="t0")

            # ---- qkv projection (V first so Vtil prep overlaps K/Q) ----
            with tc.tile_pool(name="ps_pre", bufs=2, space="PSUM") as pre:
                for jt in [2, 1, 0]:
                    for st in range(NQ):
                        pq = pre.tile([P, 512], FP, tag="mm")
                        for dc in range(DC):
                            nc.tensor.matmul(
                                pq,
                                lhsT=w_sb[:, dc, jt * P : (jt + 1) * P],
                                rhs=xT[:, dc, st * 512 : (st + 1) * 512],
                                start=(dc == 0),
                                stop=(dc == DC - 1),
                            )
                        if jt == 2:
                            dst = vT[:, st * 512 : (st + 1) * 512]
                        else:
                            dst = qkvT[:, jt, st * 512 : (st + 1) * 512]
                        if bias_sb is not None:
                            nc.vector.tensor_scalar_add(
                                dst, pq, bias_sb[:, jt : jt + 1]
                            )
                        else:
                            nc.scalar.copy(dst, pq)
                    if jt == 2:
                        # Vtil: V with k on partitions, per head:
                        # [V(64 cols) | mask/ones col]
                        for h in range(H):
                            if mask_sb is not None:
                                nc.vector.tensor_copy(
                                    vt[h][:, :, DH : DH + 1], mask_sb[:, :, None]
                                )
                            else:
                                nc.vector.memset(vt[h][:, :, DH : DH + 1], 1.0)
                        for c in range(SC):
                            pt = pre.tile([P, P], BF, tag="tr")
                            nc.tensor.transpose(
                                pt, vT[:, c * P : (c + 1) * P], ident
                            )
                            for h in range(H):
                                if mask_sb is not None:
                                    nc.scalar.activation(
                                        vt[h][:, c, 0:DH],
                                        pt[:, h * DH : (h + 1) * DH],
                                        AF.Copy,
                                        scale=mask_sb[:, c : c + 1],
                                    )
                                else:
                                    nc.scalar.copy(
                                        vt[h][:, c, 0:DH],
                                        pt[:, h * DH : (h + 1) * DH],
                                    )

            # ---- attention: 4 units (h, q-half), tails overlapped ----
            y_t3 = y_d.rearrange("(c p) m -> p c m", p=P)

            with (
                tc.tile_pool(name="ps_sc", bufs=2, space="PSUM") as psc,
                tc.tile_pool(name="ps_pv", bufs=1, space="PSUM") as ppv,
                tc.tile_pool(name="ps_tl", bufs=2, space="PSUM") as ptl,
                tc.tile_pool(name="expp", bufs=10) as expp,
                tc.tile_pool(name="post", bufs=8) as post,
            ):

                def make_tail(h, qh, pv):
                    """Return list of closures emitting unit (h, qh)'s tail."""
                    u = h * 2 + qh
                    q0 = qh * 1024
                    # per-unit LN state (h==1 units only)
                    mv_u = post.tile([P, 16], FP, tag="mv_u", name=f"mv{u}")
                    yts = [None] * 8
                    rstd_u = post.tile([P, 8], FP, tag="rstd", name=f"rst{u}")

                    def head():
                        nc.scalar.copy(
                            attnT[h][0:DH, q0 : q0 + 1024], pv[0:DH, :]
                        )
                        nc.scalar.copy(
                            stage[DH : DH + 1, h * S + q0 : h * S + q0 + 1024],
                            pv[DH : DH + 1, :],
                        )

                    def denoms():
                        dn = ptl.tile([P, 512], FP, tag="tl", name=f"dn{u}")
                        for j in range(8):
                            c8 = qh * 8 + j
                            nc.tensor.transpose(
                                dn[:, j : j + 1],
                                stage[
                                    DH : DH + 1,
                                    h * S + c8 * P : h * S + (c8 + 1) * P,
                                ],
                                one_f[DH : DH + 1, 0:1],
                            )
                        nc.vector.reciprocal(
                            r_sb[:, u * 8 : u * 8 + 8], dn[:, 0:8]
                        )

                    def chunk(j):
                        c8 = qh * 8 + j
                        rsc = r_sb[:, u * 8 + j : u * 8 + j + 1]
                        po = ptl.tile([P, 512], FP, tag="tl", name=f"po{u}_{j}")
                        nc.tensor.matmul(
                            po[:, 0:DM],
                            lhsT=attnT[h][:, c8 * P : (c8 + 1) * P],
                            rhs=ow_sb[:, h, :],
                            start=True,
                            stop=True,
                        )
                        if h == 0:
                            nc.vector.scalar_tensor_tensor(
                                t0[:, c8, :], po[:, 0:DM], rsc, xr[:, c8, :],
                                op0=OP.mult, op1=OP.add,
                            )
                        else:
                            y_t = post.tile([P, DM], FP, tag="y_t",
                                            name=f"y{u}_{j}")
                            yts[j] = y_t
                            nc.vector.scalar_tensor_tensor(
                                y_t, po[:, 0:DM], rsc, t0[:, c8, :],
                                op0=OP.mult, op1=OP.add,
                            )
                            st_t = post.tile([P, 6], FP, tag="st")
                            nc.vector.bn_stats(st_t, y_t)
                            nc.vector.bn_aggr(mv_u[:, 2 * j : 2 * j + 2], st_t)

                    def rsqrt():
                        # rstd = 1/sqrt(var+eps) via Quake seed + 2 Newton
                        # steps, batched [P,8] — keeps Sqrt off ACT (avoids
                        # Exp<->Sqrt activation-table reloads)
                        veps = post.tile([P, 8], FP, tag="veps")
                        nc.vector.tensor_scalar_add(
                            veps, mv_u[:, 1:16:2], LN_EPS
                        )
                        shi = post.tile([P, 8], I32, tag="shi")
                        nc.vector.tensor_scalar(
                            shi, veps[:, :].bitcast(I32), 1, None,
                            op0=OP.arith_shift_right,
                        )
                        y0i = post.tile([P, 8], I32, tag="y0i")
                        nc.vector.tensor_tensor(
                            y0i, ctile, shi, op=OP.subtract
                        )
                        x2 = post.tile([P, 8], FP, tag="x2")
                        nc.vector.tensor_scalar_mul(x2, veps, 0.5)
                        y = y0i[:, :].bitcast(FP)
                        for it in range(2):
                            aa = post.tile([P, 8], FP, tag=f"aa{it}")
                            nc.vector.tensor_tensor(aa, y, y, op=OP.mult)
                            nc.vector.tensor_tensor(aa, x2, aa, op=OP.mult)
                            nc.vector.tensor_scalar(
                                aa, aa, -1.0, 1.5, op0=OP.mult, op1=OP.add
                            )
                            dst = rstd_u if it == 1 else post.tile(
                                [P, 8], FP, tag="yn"
                            )
                            nc.vector.tensor_tensor(dst, y, aa, op=OP.mult)
                            y = dst

                    def norm(j):
                        c8 = qh * 8 + j
                        o_t = post.tile([P, DM], FP, tag="o_t")
                        nc.vector.tensor_scalar(
                            o_t, yts[j],
                            scalar1=mv_u[:, 2 * j : 2 * j + 1],
                            scalar2=rstd_u[:, j : j + 1],
                            op0=OP.subtract, op1=OP.mult,
                        )
                        if g_sb is not None and b_sb is not None:
                            nc.vector.tensor_mul(o_t, o_t, g_sb)
                            nc.vector.tensor_add(o_t, o_t, b_sb)
                        nc.sync.dma_start(y_t3[:, c8, :], o_t)

                    head()
                    pieces = [denoms] + [
                        (lambda j=j: chunk(j)) for j in range(8)
                    ]
                    if h == 1:
                        pieces.append(rsqrt)
                        pieces += [(lambda j=j: norm(j)) for j in range(8)]
                    return pieces

                tail_work: list = []
                for h in range(H):
                    hs = slice(h * DH, (h + 1) * DH)
                    for qh in range(2):
                        q_base = qh * 1024
                        pv = ppv.tile([P, 1024], FP, tag="pv", name=f"pv{h}_{qh}")

                        def emit_pv(pend):
                            pr, expair = pend
                            for qq in range(2):
                                nc.tensor.matmul(
                                    pv[0 : DH + 1, qq * 512 : (qq + 1) * 512],
                                    lhsT=vt[h][:, 2 * pr : 2 * pr + 2, 0 : DH + 1],
                                    rhs=expair[:, :, qq * 512 : (qq + 1) * 512],
                                    start=(pr == 0),
                                    stop=(pr == SC // 2 - 1),
                                    perf_mode=DROW,
                                )

                        pending = []
                        expair = None
                        for c in range(SC):
                            pr, half = divmod(c, 2)
                            sc_ps = psc.tile([P, 1024], FP, tag="sc")
                            for qq in range(2):
                                q0 = q_base + qq * 512
                                nc.tensor.matmul(
                                    sc_ps[:, qq * 512 : (qq + 1) * 512],
                                    lhsT=qkvT[hs, 1, c * P : (c + 1) * P],
                                    rhs=qkvT[hs, 0, q0 : q0 + 512],
                                    start=True,
                                    stop=True,
                                )
                            if half == 0:
                                expair = expp.tile(
                                    [P, 2, 1024], F8, tag="expair"
                                )
                            # alternate exp between ACT (exact) and DVE
                            # (Schraudolph int8 in e4m3 space)
                            if c % 2 == 0:
                                nc.scalar.activation(
                                    expair[:, half, :], sc_ps, AF.Exp,
                                    scale=SCALE,
                                )
                            else:
                                nc.vector.tensor_scalar(
                                    expair[:, half, :].bitcast(I8), sc_ps,
                                    A_SCHR8, B_SCHR8, op0=OP.mult, op1=OP.add,
                                )
                            if half == 1:
                                pending.append((pr, expair))
                                if len(pending) > 2:
                                    emit_pv(pending.pop(0))
                            if c >= 2 and tail_work:
                                tail_work.pop(0)()
                        for pend in pending:
                            emit_pv(pend)
                        # flush any tail remnants before reusing pv psum
                        while tail_work:
                            tail_work.pop(0)()
                        tail_work = make_tail(h, qh, pv)
                for piece in tail_work:
                    piece()

    nc.compile()
    return nc


_PROGRAM_CACHE: dict = {}


def _get_program(key):
    if key not in _PROGRAM_CACHE:
        _PROGRAM_CACHE[key] = _build(*key)
    return _PROGRAM_CACHE[key]


def kernel(x, attn_mask, qkv_w, qkv_b, o_w, ln_g, ln_b, **_ignored):
    x = np.ascontiguousarray(np.asarray(x, dtype=np.float32))
    attn_mask = np.asarray(attn_mask)
    qkv_w = np.ascontiguousarray(np.asarray(qkv_w, dtype=np.float32))
    qkv_b = np.asarray(qkv_b, dtype=np.float32)
    o_w = np.ascontiguousarray(np.asarray(o_w, dtype=np.float32))
    ln_g = np.asarray(ln_g, dtype=np.float32)
    ln_b = np.asarray(ln_b, dtype=np.float32)

    has_mask = not bool(attn_mask.all())
    has_bias = bool(np.any(qkv_b != 0.0))
    has_affine = bool(np.any(ln_g != 1.0) or np.any(ln_b != 0.0))

    nc = _get_program((has_mask, has_bias, has_affine))

    mask_f = attn_mask.astype(np.float32)
    in_maps = []
    for i in range(N_CORES):
        m = {
            "x_bf": np.ascontiguousarray(x[i].astype(ml_dtypes.bfloat16)),
            "qkv_w_bf": qkv_w.astype(ml_dtypes.bfloat16),
            "o_w_bf": o_w.astype(ml_dtypes.bfloat16),
        }
        if has_mask:
            m["mask_f"] = np.ascontiguousarray(mask_f[i])
        if has_bias:
            m["qkv_b"] = qkv_b
        if has_affine:
            m["ln_g"] = ln_g
            m["ln_b"] = ln_b
        in_maps.append(m)

    trace = os.environ.get("KBENCH_TRACE", "0") == "1"
    kw = {}
    if trace:
        kw = {"trace": True, "trace_cores": [0]}
    res = run_bass_kernel_spmd(nc, in_maps, core_ids=list(range(N_CORES)), **kw)
    global LAST_RESULT
    LAST_RESULT = res
    return np.stack([res.results[i]["y"] for i in range(N_CORES)], axis=0)


LAST_RESULT = None


# revision 44
# speedup vs baseline: 1.0214x; 1.0214x over previous
"""Trainium2 Bass kernel for a 2-head MultiHeadAttn + residual + LayerNorm block.

Problem shapes (hardcoded):
  x:      [8, 2048, 384] f32      attn_mask: [8, 2048] bool (True = attend)
  qkv_w:  [384, 384] f32          qkv_b: [384] f32
  o_w:    [128, 384] f32          ln_g, ln_b: [384] f32
  out:    [8, 2048, 384] f32

Sharding: data-parallel over batch — 8 batch elements, one per NeuronCore.
Each core runs the identical program (SPMD) on its own batch slice.

Per-core dataflow (S=2048, D_model=384, H=2, Dh=64):
  1. Weights DMA first (qkv needs them immediately), then xT [128, 3, 2048]
     bf16 via DMA-transpose split across the two HWDGE queues (sync +
     scalar), then the residual copy xr [128, 16, 384] bf16.
  2. qkvT = (x @ qkv_w).T as [128 j, 2048 s] in fp8e4; j-tile order V,K,Q
     so V-prep (per-head fp8 Vtil with a ones/mask column appended for the
     softmax denominator) overlaps the K/Q projections.
  3. Attention in 4 units (head h x q-half): per k-chunk c, scores^T
     [128 k, 1024 q] on PE (fp8 Q/K); exp alternates per chunk between ACT
     (exact Exp, fp8e4 out) and DVE (Schraudolph bit-trick:
     round(s*SCALE*8/ln2 + 56) as int8 = e4m3 bits, ~5% sawtooth error that
     cancels in softmax normalization); pv uses fp8 DoubleRow matmuls
     (contraction 256 = two k-chunks per pass) accumulating [65, 1024]
     (row 64 = denominator).
  4. Each unit's tail (denominator transpose, reciprocal, o-projection,
     residual combine, LayerNorm) is interleaved piecewise into the NEXT
     unit's attention loop so PE/ACT/DVE all stay dense.  rstd comes from a
     batched Quake rsqrt + 2 Newton steps on DVE (keeps Sqrt off ACT,
     avoiding Exp<->Sqrt activation-table reload thrash).
"""

import os
import sys

import ml_dtypes
import numpy as np

for _p in ("/opt/trn_rl_repo", "/root/.axon_site/_ro/trn_rl_repo"):
    if os.path.isdir(_p) and _p not in sys.path:
        sys.path.insert(0, _p)

import concourse.bass as bass  # noqa: E402
import concourse.tile as tile  # noqa: E402
from concourse import bacc  # noqa: E402
from concourse import mybir  # noqa: E402
from concourse.bass_utils import run_bass_kernel_spmd  # noqa: E402

FP = mybir.dt.float32
BF = mybir.dt.bfloat16
I16 = mybir.dt.int16
I8 = mybir.dt.int8
I32 = mybir.dt.int32
F8 = mybir.dt.float8e4
AF = mybir.ActivationFunctionType
OP = mybir.AluOpType
DROW = mybir.MatmulPerfMode.DoubleRow

B, S, DM = 8, 2048, 384
H, DH = 2, 64
INNER = H * DH  # 128
P = 128
SC = S // P  # 16 s-chunks of 128
DC = DM // P  # 3 model-dim chunks of 128
NQ = S // 512  # 4 q-tiles of 512
LN_EPS = 1e-3
N_CORES = 8
SCALE = 1.0 / (DH**0.5)
# Schraudolph exp in fp8e4-space: exp(SCALE*s) ~ bitcast_e4m3(int8(
#   round(SCALE*s * 8/ln2 + 7*8)))
A_SCHR8 = SCALE * 8.0 / float(np.log(2.0))
B_SCHR8 = 7.0 * 8.0
QUAKE_C = 0x5F3759DF  # rsqrt seed constant


def _build(has_mask: bool, has_bias: bool, has_affine: bool) -> bass.Bass:
    nc = bacc.Bacc(
        "TRN2", target_bir_lowering=False, debug=False, num_devices=N_CORES
    )

    xb_d = nc.dram_tensor("x_bf", [S, DM], BF, kind="ExternalInput")
    w_d = nc.dram_tensor("qkv_w_bf", [DM, 3 * INNER], BF, kind="ExternalInput")
    ow_d = nc.dram_tensor("o_w_bf", [INNER, DM], BF, kind="ExternalInput")
    mask_d = bias_d = g_d = b_d = None
    if has_mask:
        mask_d = nc.dram_tensor("mask_f", [S], FP, kind="ExternalInput")
    if has_bias:
        bias_d = nc.dram_tensor("qkv_b", [3 * INNER], FP, kind="ExternalInput")
    if has_affine:
        g_d = nc.dram_tensor("ln_g", [DM], FP, kind="ExternalInput")
        b_d = nc.dram_tensor("ln_b", [DM], FP, kind="ExternalInput")
    y_d = nc.dram_tensor("y", [S, DM], FP, kind="ExternalOutput")

    with tile.TileContext(nc) as tc:
        with tc.tile_pool(name="singles", bufs=1) as sg:
            # ---- input DMAs; weights first (qkv needs them immediately),
            # then xT transposes split across both HWDGE queues ----
            tq = [nc.sync, nc.scalar]
            w_sb = sg.tile([P, DC, 3 * INNER], BF, tag="w_sb")
            nc.sync.dma_start(w_sb, w_d.rearrange("(dc dp) j -> dp dc j", dp=P))
            ow_sb = sg.tile([DH, H, DM], BF, tag="ow_sb")
            nc.scalar.dma_start(ow_sb, ow_d.rearrange("(h d) m -> d h m", d=DH))
            xT = sg.tile([P, DC, S], BF, tag="xT")
            for st in range(NQ):
                for dc in range(DC):
                    tq[(st * DC + dc) % 2].dma_start_transpose(
                        xT[:, dc, st * 512 : (st + 1) * 512],
                        xb_d[st * 512 : (st + 1) * 512, dc * P : (dc + 1) * P],
                    )
            xr = sg.tile([P, SC, DM], BF, tag="xr")
            xr_src = xb_d.rearrange("(c p) d -> p c d", p=P)
            for c in range(SC):
                tq[c % 2].dma_start(xr[:, c, :], xr_src[:, c, :])

            from concourse.masks import make_identity
            ident = sg.tile([P, P], BF, tag="ident")
            make_identity(nc, ident)
            one_f = sg.tile([P, 1], FP, tag="one_f")
            nc.vector.memset(one_f, 1.0)
            ctile = sg.tile([P, 8], I32, tag="ctile")
            nc.vector.memset(ctile, QUAKE_C)

            mask_sb = bias_sb = g_sb = b_sb = None
            if mask_d is not None:
                mask_sb = sg.tile([P, SC], FP, tag="mask_sb")
                nc.sync.dma_start(mask_sb, mask_d.rearrange("(c p) -> p c", p=P))
            if bias_d is not None:
                bias_sb = sg.tile([P, 3], FP, tag="bias_sb")
                nc.sync.dma_start(bias_sb, bias_d.rearrange("(jt p) -> p jt", p=P))
            if g_d is not None and b_d is not None:
                g_sb = sg.tile([P, DM], FP, tag="g_sb")
                b_sb = sg.tile([P, DM], FP, tag="b_sb")
                nc.gpsimd.dma_start(g_sb, g_d[None, :].to_broadcast((P, DM)))
                nc.gpsimd.dma_start(b_sb, b_d[None, :].to_broadcast((P, DM)))

            qkvT = sg.tile([P, 2, S], F8, tag="qkvT")  # j-tile: 0=Q^T 1=K^T
            vT = sg.tile([P, S], BF, tag="vT")
            vt = [
                sg.tile([P, SC, 80], F8, tag=f"vt{h}", name=f"vt{h}")
                for h in range(H)
            ]
            attnT = [
                sg.tile([DH, S], BF, tag=f"attnT{h}", name=f"attnT{h}")
                for h in range(H)
            ]
            stage = sg.tile([P, H * S], FP, tag="stage")
            r_sb = sg.tile([P, H * SC], FP, tag="r_sb")
            t0 = sg.tile([P, SC, DM], FP, tag="t0")

            # ---- qkv projection; st-outer so each q-range starts as soon
            # as its 3 xT transpose chunks land (jt order V,K,Q within) ----
            with tc.tile_pool(name="ps_pre", bufs=2, space="PSUM") as pre:
                for h in range(H):
                    if mask_sb is not None:
                        nc.vector.tensor_copy(
                            vt[h][:, :, DH : DH + 1], mask_sb[:, :, None]
                        )
                    else:
                        nc.vector.memset(vt[h][:, :, DH : DH + 1], 1.0)
                for st in range(NQ):
                    for jt in [2, 1, 0]:
                        pq = pre.tile([P, 512], FP, tag="mm")
                        for dc in range(DC):
                            nc.tensor.matmul(
                                pq,
                                lhsT=w_sb[:, dc, jt * P : (jt + 1) * P],
                                rhs=xT[:, dc, st * 512 : (st + 1) * 512],
                                start=(dc == 0),
                                stop=(dc == DC - 1),
                            )
                        if jt == 2:
                            dst = vT[:, st * 512 : (st + 1) * 512]
                        else:
                            dst = qkvT[:, jt, st * 512 : (st + 1) * 512]
                        if bias_sb is not None:
                            nc.vector.tensor_scalar_add(
                                dst, pq, bias_sb[:, jt : jt + 1]
                            )
                        else:
                            nc.scalar.copy(dst, pq)
                # Vtil prep AFTER all projections: the PE transposes here
                # are paced by the ACT copies, so they must not sit in
                # front of the K/Q matmuls
                for c in range(SC):
                    pt = pre.tile([P, P], BF, tag="tr")
                    nc.tensor.transpose(pt, vT[:, c * P : (c + 1) * P], ident)
                    for h in range(H):
                        if mask_sb is not None:
                            nc.scalar.activation(
                                vt[h][:, c, 0:DH],
                                pt[:, h * DH : (h + 1) * DH],
                                AF.Copy,
                                scale=mask_sb[:, c : c + 1],
                            )
                        else:
                            nc.scalar.copy(
                                vt[h][:, c, 0:DH],
                                pt[:, h * DH : (h + 1) * DH],
                            )

            # ---- attention: 4 units (h, q-half), tails overlapped ----
            y_t3 = y_d.rearrange("(c p) m -> p c m", p=P)

            with (
                tc.tile_pool(name="ps_sc", bufs=2, space="PSUM") as psc,
                tc.tile_pool(name="ps_pv", bufs=1, space="PSUM") as ppv,
                tc.tile_pool(name="ps_tl", bufs=2, space="PSUM") as ptl,
                tc.tile_pool(name="expp", bufs=10) as expp,
                tc.tile_pool(name="post", bufs=8) as post,
            ):

                def make_tail(h, qh, pv):
                    """Return list of closures emitting unit (h, qh)'s tail."""
                    u = h * 2 + qh
                    q0 = qh * 1024
                    # per-unit LN state (h==1 units only)
                    mv_u = post.tile([P, 16], FP, tag="mv_u", name=f"mv{u}")
                    yts = [None] * 8
                    rstd_u = post.tile([P, 8], FP, tag="rstd", name=f"rst{u}")

                    def head():
                        nc.scalar.copy(
                            attnT[h][0:DH, q0 : q0 + 1024], pv[0:DH, :]
                        )
                        nc.scalar.copy(
                            stage[DH : DH + 1, h * S + q0 : h * S + q0 + 1024],
                            pv[DH : DH + 1, :],
                        )

                    def denoms():
                        dn = ptl.tile([P, 512], FP, tag="tl", name=f"dn{u}")
                        for j in range(8):
                            c8 = qh * 8 + j
                            nc.tensor.transpose(
                                dn[:, j : j + 1],
                                stage[
                                    DH : DH + 1,
                                    h * S + c8 * P : h * S + (c8 + 1) * P,
                                ],
                                one_f[DH : DH + 1, 0:1],
                            )
                        nc.vector.reciprocal(
                            r_sb[:, u * 8 : u * 8 + 8], dn[:, 0:8]
                        )

                    def chunk(j):
                        c8 = qh * 8 + j
                        rsc = r_sb[:, u * 8 + j : u * 8 + j + 1]
                        po = ptl.tile([P, 512], FP, tag="tl", name=f"po{u}_{j}")
                        nc.tensor.matmul(
                            po[:, 0:DM],
                            lhsT=attnT[h][:, c8 * P : (c8 + 1) * P],
                            rhs=ow_sb[:, h, :],
                            start=True,
                            stop=True,
                        )
                        if h == 0:
                            nc.vector.scalar_tensor_tensor(
                                t0[:, c8, :], po[:, 0:DM], rsc, xr[:, c8, :],
                                op0=OP.mult, op1=OP.add,
                            )
                        else:
                            y_t = post.tile([P, DM], FP, tag="y_t",
                                            name=f"y{u}_{j}")
                            yts[j] = y_t
                            nc.vector.scalar_tensor_tensor(
                                y_t, po[:, 0:DM], rsc, t0[:, c8, :],
                                op0=OP.mult, op1=OP.add,
                            )
                            st_t = post.tile([P, 6], FP, tag="st")
                            nc.vector.bn_stats(st_t, y_t)
                            nc.vector.bn_aggr(mv_u[:, 2 * j : 2 * j + 2], st_t)

                    def rsqrt():
                        # rstd = 1/sqrt(var+eps) via Quake seed + 2 Newton
                        # steps, batched [P,8] — keeps Sqrt off ACT (avoids
                        # Exp<->Sqrt activation-table reloads)
                        veps = post.tile([P, 8], FP, tag="veps")
                        nc.vector.tensor_scalar_add(
                            veps, mv_u[:, 1:16:2], LN_EPS
                        )
                        shi = post.tile([P, 8], I32, tag="shi")
                        nc.vector.tensor_scalar(
                            shi, veps[:, :].bitcast(I32), 1, None,
                            op0=OP.arith_shift_right,
                        )
                        y0i = post.tile([P, 8], I32, tag="y0i")
                        nc.vector.tensor_tensor(
                            y0i, ctile, shi, op=OP.subtract
                        )
                        x2 = post.tile([P, 8], FP, tag="x2")
                        nc.vector.tensor_scalar_mul(x2, veps, 0.5)
                        y = y0i[:, :].bitcast(FP)
                        for it in range(2):
                            aa = post.tile([P, 8], FP, tag=f"aa{it}")
                            nc.vector.tensor_tensor(aa, y, y, op=OP.mult)
                            nc.vector.tensor_tensor(aa, x2, aa, op=OP.mult)
                            nc.vector.tensor_scalar(
                                aa, aa, -1.0, 1.5, op0=OP.mult, op1=OP.add
                            )
                            dst = rstd_u if it == 1 else post.tile(
                                [P, 8], FP, tag="yn"
                            )
                            nc.vector.tensor_tensor(dst, y, aa, op=OP.mult)
                            y = dst

                    def norm(j):
                        c8 = qh * 8 + j
                        o_t = post.tile([P, DM], FP, tag="o_t")
                        nc.vector.tensor_scalar(
                            o_t, yts[j],
                            scalar1=mv_u[:, 2 * j : 2 * j + 1],
                            scalar2=rstd_u[:, j : j + 1],
                            op0=OP.subtract, op1=OP.mult,
                        )
                        if g_sb is not None and b_sb is not None:
                            nc.vector.tensor_mul(o_t, o_t, g_sb)
                            nc.vector.tensor_add(o_t, o_t, b_sb)
                        nc.sync.dma_start(y_t3[:, c8, :], o_t)

                    head()
                    pieces = [denoms] + [
                        (lambda j=j: chunk(j)) for j in range(8)
                    ]
                    if h == 1:
                        pieces.append(rsqrt)
                        pieces += [(lambda j=j: norm(j)) for j in range(8)]
                    return pieces

                tail_work: list = []
                for h in range(H):
                    hs = slice(h * DH, (h + 1) * DH)
                    for qh in range(2):
                        q_base = qh * 1024
                        pv = ppv.tile([P, 1024], FP, tag="pv", name=f"pv{h}_{qh}")

                        def emit_pv(pend):
                            pr, expair = pend
                            for qq in range(2):
                                nc.tensor.matmul(
                                    pv[0 : DH + 1, qq * 512 : (qq + 1) * 512],
                                    lhsT=vt[h][:, 2 * pr : 2 * pr + 2, 0 : DH + 1],
                                    rhs=expair[:, :, qq * 512 : (qq + 1) * 512],
                                    start=(pr == 0),
                                    stop=(pr == SC // 2 - 1),
                                    perf_mode=DROW,
                                )

                        pending = []
                        expair = None
                        for c in range(SC):
                            pr, half = divmod(c, 2)
                            sc_ps = psc.tile([P, 1024], FP, tag="sc")
                            for qq in range(2):
                                q0 = q_base + qq * 512
                                nc.tensor.matmul(
                                    sc_ps[:, qq * 512 : (qq + 1) * 512],
                                    lhsT=qkvT[hs, 1, c * P : (c + 1) * P],
                                    rhs=qkvT[hs, 0, q0 : q0 + 512],
                                    start=True,
                                    stop=True,
                                )
                            if half == 0:
                                expair = expp.tile(
                                    [P, 2, 1024], F8, tag="expair"
                                )
                            # alternate exp between ACT (exact) and DVE
                            # (Schraudolph int8 in e4m3 space)
                            if c % 2 == 0:
                                nc.scalar.activation(
                                    expair[:, half, :], sc_ps, AF.Exp,
                                    scale=SCALE,
                                )
                            else:
                                nc.vector.tensor_scalar(
                                    expair[:, half, :].bitcast(I8), sc_ps,
                                    A_SCHR8, B_SCHR8, op0=OP.mult, op1=OP.add,
                                )
                            if half == 1:
                                pending.append((pr, expair))
                                if len(pending) > 2:
                                    emit_pv(pending.pop(0))
                            if c >= 2 and tail_work:
                                tail_work.pop(0)()
                        for pend in pending:
                            emit_pv(pend)
                        # flush any tail remnants before reusing pv psum
                        while tail_work:
                            tail_work.pop(0)()
                        tail_work = make_tail(h, qh, pv)
                for piece in tail_work:
                    piece()

    nc.compile()
    return nc


_PROGRAM_CACHE: dict = {}


def _get_program(key):
    if key not in _PROGRAM_CACHE:
        _PROGRAM_CACHE[key] = _build(*key)
    return _PROGRAM_CACHE[key]


def kernel(x, attn_mask, qkv_w, qkv_b, o_w, ln_g, ln_b, **_ignored):
    x = np.ascontiguousarray(np.asarray(x, dtype=np.float32))
    attn_mask = np.asarray(attn_mask)
    qkv_w = np.ascontiguousarray(np.asarray(qkv_w, dtype=np.float32))
    qkv_b = np.asarray(qkv_b, dtype=np.float32)
    o_w = np.ascontiguousarray(np.asarray(o_w, dtype=np.float32))
    ln_g = np.asarray(ln_g, dtype=np.float32)
    ln_b = np.asarray(ln_b, dtype=np.float32)

    has_mask = not bool(attn_mask.all())
    has_bias = bool(np.any(qkv_b != 0.0))
    has_affine = bool(np.any(ln_g != 1.0) or np.any(ln_b != 0.0))

    nc = _get_program((has_mask, has_bias, has_affine))

    mask_f = attn_mask.astype(np.float32)
    in_maps = []
    for i in range(N_CORES):
        m = {
            "x_bf": np.ascontiguousarray(x[i].astype(ml_dtypes.bfloat16)),
            "qkv_w_bf": qkv_w.astype(ml_dtypes.bfloat16),
            "o_w_bf": o_w.astype(ml_dtypes.bfloat16),
        }
        if has_mask:
            m["mask_f"] = np.ascontiguousarray(mask_f[i])
        if has_bias:
            m["qkv_b"] = qkv_b
        if has_affine:
            m["ln_g"] = ln_g
            m["ln_b"] = ln_b
        in_maps.append(m)

    trace = os.environ.get("KBENCH_TRACE", "0") == "1"
    kw = {}
    if trace:
        kw = {"trace": True, "trace_cores": [0]}
    res = run_bass_kernel_spmd(nc, in_maps, core_ids=list(range(N_CORES)), **kw)
    global LAST_RESULT
    LAST_RESULT = res
    return np.stack([res.results[i]["y"] for i in range(N_CORES)], axis=0)


LAST_RESULT = None
